# revision 2
# baseline (speedup 1.0000x reference)
"""Trainium2 Bass kernel for the attention-mechanism (GMM window attention) module.

Computation (per batch row b):
  raw = x @ W + bias                       [B, 30] -> alpha_hat|beta_hat|kappa_hat
  alpha = exp(clip(alpha_hat, -8, 8))
  beta  = exp(clip(beta_hat, -8, 8))
  kappa = prev_kappa + exp(clip(kappa_hat + kappa_scale, -8, 5))
  phi[b,l]  = sum_k alpha_k * exp(clip(-beta_k*(kappa_k-(l+1))^2, -50, 0))
  phi      *= (l < seq_len[b])
  w[b,v]    = sum_l phi[b,l] * one_hot[b,l,v]
Returns (w [B,V], kappa [B,K]).

Strategy: pure data parallel over 8 cores (512 rows each). Per core:
  - phi computed with batch rows on partitions (ACT exp, DVE elementwise).
  - phi transposed to [l, b] layout via PE transposes.
  - The big einsum runs on the PE: per batch row, one_hot chunk [128l, 80v]
    is the stationary operand, the phi column [128l, 1] is the moving
    operand; out [80, 1] accumulates into psum[0:80, b].  The one_hot
    tensor (335 MB total) is streamed once from HBM -> memory bound.
"""

import sys

sys.path.insert(0, "/opt/trn_rl_repo")

import numpy as np

import concourse.bass as bass
import concourse.bacc as bacc
import concourse.tile as tile
from concourse import mybir
from concourse.bass_utils import run_bass_kernel_spmd
from concourse.masks import make_identity

F32 = mybir.dt.float32
I32 = mybir.dt.int32
AF = mybir.ActivationFunctionType
OP = mybir.AluOpType

B, D, K, L, V = 4096, 512, 10, 256, 80
K3 = 3 * K
NCORES = 8
BC = B // NCORES          # 512 rows per core
NT = BC // 128            # 4 batch tiles per core

_CACHE = {}


def build_nc():
    nc = bacc.Bacc(
        "TRN2",
        target_bir_lowering=False,
        debug=False,
        num_devices=NCORES,
    )

    x = nc.dram_tensor("x", [BC, D], F32, kind="ExternalInput").ap()
    pk = nc.dram_tensor("pk", [BC, K], F32, kind="ExternalInput").ap()
    oh = nc.dram_tensor("oh", [BC, L, V], F32, kind="ExternalInput").ap()
    slen = nc.dram_tensor("slen", [BC, 1], I32, kind="ExternalInput").ap()
    Wd = nc.dram_tensor("W", [D, K3], F32, kind="ExternalInput").ap()
    bv = nc.dram_tensor("b", [1, K3], F32, kind="ExternalInput").ap()
    ks = nc.dram_tensor("ks", [1, 1], F32, kind="ExternalInput").ap()
    wT = nc.dram_tensor("wT", [V, BC], F32, kind="ExternalOutput").ap()
    kap = nc.dram_tensor("kap", [BC, K], F32, kind="ExternalOutput").ap()

    with tile.TileContext(nc) as tc:
        with (
            tc.tile_pool(name="singles", bufs=1) as singles,
            tc.tile_pool(name="ptile", bufs=2) as ptile,
            tc.tile_pool(name="small", bufs=2) as small,
            tc.tile_pool(name="kwork", bufs=3) as kwork,
            tc.tile_pool(name="ohp", bufs=12) as ohp,
            tc.tile_pool(name="pst", bufs=2, space="PSUM") as pst,
            tc.tile_pool(name="psr", bufs=2, space="PSUM") as psr,
            tc.tile_pool(name="psw", bufs=1, space="PSUM") as psw,
        ):
            # ---- constants ----
            ident = singles.tile([128, 128], F32)
            make_identity(nc, ident)

            u0i = singles.tile([128, L], I32)
            nc.gpsimd.iota(u0i, pattern=[[1, L]], base=0, channel_multiplier=0)
            u0 = singles.tile([128, L], F32)
            nc.vector.tensor_copy(u0, u0i)  # 0..255 on every partition

            W_sb = singles.tile([128, D // 128, K3], F32)
            nc.sync.dma_start(out=W_sb, in_=Wd.rearrange("(c p) k -> p c k", p=128))

            b_bc = singles.tile([128, K3], F32)
            nc.gpsimd.dma_start(
                out=b_bc,
                in_=bass.AP(tensor=bv.tensor, offset=0, ap=[[0, 128], [1, K3]]),
            )
            ks_col = singles.tile([128, 1], F32)
            nc.gpsimd.dma_start(
                out=ks_col,
                in_=bass.AP(tensor=ks.tensor, offset=0, ap=[[0, 128], [1, 1]]),
            )
            # fold kappa_scale into the kappa_hat slice of the bias
            nc.vector.tensor_scalar(
                out=b_bc[:, 2 * K : 3 * K],
                in0=b_bc[:, 2 * K : 3 * K],
                scalar1=ks_col[:],
                scalar2=None,
                op0=OP.add,
            )

            phiTe = singles.tile([128, BC], F32)  # phi[b, 2p]   (even l)
            phiTo = singles.tile([128, BC], F32)  # phi[b, 2p+1] (odd l)

            # ---- phase A: phi / kappa per 128-row batch tile ----
            for t in range(NT):
                row = slice(t * 128, (t + 1) * 128)

                x_t = ptile.tile([128, D], F32, tag="x_t")
                nc.sync.dma_start(out=x_t, in_=x[row, :])
                pk_t = small.tile([128, K], F32, tag="pk_t")
                nc.sync.dma_start(out=pk_t, in_=pk[row, :])
                len_i = small.tile([128, 1], I32, tag="len_i")
                nc.sync.dma_start(out=len_i, in_=slen[row, :])
                len_f = small.tile([128, 1], F32, tag="len_f")
                nc.vector.tensor_copy(len_f, len_i)

                # x tile transposed: xT[d, b] in 4 chunks of 128
                xT = ptile.tile([128, D // 128, 128], F32, tag="xT")
                for dc in range(D // 128):
                    ps = pst.tile([128, 128], F32, tag="ps_tr")
                    nc.tensor.transpose(ps, x_t[:, dc * 128 : (dc + 1) * 128], ident)
                    nc.vector.tensor_copy(xT[:, dc, :], ps)

                # raw = x @ W  (accumulate over 4 d-chunks)
                ps_raw = psr.tile([128, K3], F32, tag="ps_raw")
                for dc in range(D // 128):
                    nc.tensor.matmul(
                        ps_raw,
                        xT[:, dc, :],
                        W_sb[:, dc, :],
                        start=(dc == 0),
                        stop=(dc == D // 128 - 1),
                    )
                hats = small.tile([128, K3], F32, tag="hats")
                nc.vector.scalar_tensor_tensor(
                    out=hats, in0=ps_raw, scalar=1.0, in1=b_bc,
                    op0=OP.mult, op1=OP.add,
                )

                lnal = small.tile([128, K], F32, tag="lnal")
                nc.vector.tensor_scalar(
                    out=lnal, in0=hats[:, 0:K],
                    scalar1=-8.0, scalar2=8.0, op0=OP.max, op1=OP.min,
                )
                bcl = small.tile([128, K], F32, tag="bcl")
                nc.vector.tensor_scalar(
                    out=bcl, in0=hats[:, K : 2 * K],
                    scalar1=-8.0, scalar2=8.0, op0=OP.max, op1=OP.min,
                )
                beta = small.tile([128, K], F32, tag="beta")
                nc.scalar.activation(out=beta, in_=bcl, func=AF.Exp)
                dcl = small.tile([128, K], F32, tag="dcl")
                nc.vector.tensor_scalar(
                    out=dcl, in0=hats[:, 2 * K : 3 * K],
                    scalar1=-8.0, scalar2=5.0, op0=OP.max, op1=OP.min,
                )
                dk = small.tile([128, K], F32, tag="dk")
                nc.scalar.activation(out=dk, in_=dcl, func=AF.Exp)
                kap_t = small.tile([128, K], F32, tag="kap_t")
                nc.vector.tensor_add(kap_t, pk_t, dk)
                nc.sync.dma_start(out=kap[row, :], in_=kap_t)
                # kap1 = 1 - kappa  (so that u0 + kap1 = (l+1) - kappa ... sign
                # irrelevant after squaring)
                kap1 = small.tile([128, K], F32, tag="kap1")
                nc.vector.tensor_scalar(
                    out=kap1, in0=kap_t,
                    scalar1=-1.0, scalar2=1.0, op0=OP.mult, op1=OP.add,
                )

                # e_all[p, l, k] = exp(lnal_k - min(beta_k * (u_l - kappa_k)^2, 50))
                e_all = ptile.tile([128, L, K], F32, tag="e_all")
                for k in range(K):
                    diff = kwork.tile([128, L], F32, tag="diff")
                    nc.vector.tensor_scalar(
                        out=diff, in0=u0, scalar1=kap1[:, k : k + 1],
                        scalar2=None, op0=OP.add,
                    )
                    sq = kwork.tile([128, L], F32, tag="sq")
                    nc.vector.scalar_tensor_tensor(
                        out=sq, in0=diff, scalar=beta[:, k : k + 1], in1=diff,
                        op0=OP.mult, op1=OP.mult,
                    )
                    sc = kwork.tile([128, L], F32, tag="sc")
                    nc.vector.tensor_scalar(
                        out=sc, in0=sq, scalar1=50.0, scalar2=None, op0=OP.min,
                    )
                    nc.scalar.activation(
                        out=e_all[:, :, k], in_=sc, func=AF.Exp,
                        bias=lnal[:, k : k + 1], scale=-1.0,
                    )

                phi_r = kwork.tile([128, L], F32, tag="phi_r")
                nc.vector.tensor_reduce(
                    out=phi_r, in_=e_all, axis=mybir.AxisListType.X, op=OP.add,
                )
                msk = kwork.tile([128, L], F32, tag="msk")
                nc.vector.tensor_scalar(
                    out=msk, in0=u0, scalar1=len_f[:], scalar2=None, op0=OP.is_lt,
                )
                phi_m = kwork.tile([128, L], F32, tag="phi_m")
                nc.vector.tensor_mul(phi_m, phi_r, msk)

                # transpose phi to [l, b]; even/odd split matches the 2-l-per-
                # partition packing of the one_hot stream
                phi2 = phi_m.rearrange("p (l two) -> p two l", two=2)
                pse = pst.tile([128, 128], F32, tag="ps_tr")
                nc.tensor.transpose(pse, phi2[:, 0, :], ident)
                nc.vector.tensor_copy(phiTe[:, row], pse)
                pso = pst.tile([128, 128], F32, tag="ps_tr")
                nc.tensor.transpose(pso, phi2[:, 1, :], ident)
                nc.vector.tensor_copy(phiTo[:, row], pso)

            # ---- phase B: w^T[v, b] = sum_l phi[b, l] * oh[b, l, v] on PE ----
            oh_r = oh.rearrange("b (p two) v -> b p two v", two=2)
            ps_w = psw.tile([128, BC], F32)
            for b in range(BC):
                oht = ohp.tile([128, 2, V], F32, tag="oht")
                nc.sync.dma_start(out=oht, in_=oh_r[b])
                nc.tensor.matmul(
                    ps_w[0:V, b : b + 1], oht[:, 0, :], phiTe[:, b : b + 1],
                    start=True, stop=False,
                )
                nc.tensor.matmul(
                    ps_w[0:V, b : b + 1], oht[:, 1, :], phiTo[:, b : b + 1],
                    start=False, stop=True,
                )

            w_sb = singles.tile([V, BC], F32)
            nc.vector.tensor_copy(w_sb, ps_w[0:V, :])
            nc.sync.dma_start(out=wT, in_=w_sb)

    nc.compile()
    return nc


def _get_nc():
    if "nc" not in _CACHE:
        _CACHE["nc"] = build_nc()
    return _CACHE["nc"]


def kernel(inputs, prev_kappa, char_seq_one_hot, sequence_lengths, W, b,
           kappa_scale):
    inputs = np.asarray(inputs, dtype=np.float32)
    prev_kappa = np.asarray(prev_kappa, dtype=np.float32)
    char_seq_one_hot = np.asarray(char_seq_one_hot, dtype=np.float32)
    sequence_lengths = np.asarray(sequence_lengths, dtype=np.int32)
    W = np.ascontiguousarray(np.asarray(W, dtype=np.float32))
    b_arr = np.ascontiguousarray(np.asarray(b, dtype=np.float32).reshape(1, K3))
    ks_arr = np.ascontiguousarray(
        np.asarray(kappa_scale, dtype=np.float32).reshape(1, 1)
    )

    nc = _get_nc()
    in_maps = []
    for c in range(NCORES):
        sl = slice(c * BC, (c + 1) * BC)
        in_maps.append({
            "x": np.ascontiguousarray(inputs[sl]),
            "pk": np.ascontiguousarray(prev_kappa[sl]),
            "oh": np.ascontiguousarray(char_seq_one_hot[sl]),
            "slen": np.ascontiguousarray(sequence_lengths[sl].reshape(BC, 1)),
            "W": W,
            "b": b_arr,
            "ks": ks_arr,
        })

    res = run_bass_kernel_spmd(nc, in_maps, core_ids=list(range(NCORES)))
    w = np.concatenate([r["wT"].T for r in res.results], axis=0)
    kappa = np.concatenate([r["kap"] for r in res.results], axis=0)
    return (w, kappa)


# revision 8
# speedup vs baseline: 3.8306x; 3.8306x over previous
"""Trainium2 Bass kernel for the attention-mechanism (GMM window attention) module.

Computation (per batch row b):
  raw = x @ W + bias                       [B, 30] -> alpha_hat|beta_hat|kappa_hat
  alpha = exp(clip(alpha_hat, -8, 8))
  beta  = exp(clip(beta_hat, -8, 8))
  kappa = prev_kappa + exp(clip(kappa_hat + kappa_scale, -8, 5))
  phi[b,l]  = sum_k alpha_k * exp(clip(-beta_k*(kappa_k-(l+1))^2, -50, 0))
  phi      *= (l < seq_len[b])
  w[b,v]    = sum_l phi[b,l] * one_hot[b,l,v]
Returns (w [B,V], kappa [B,K]).

Strategy: pure data parallel over 8 cores (512 rows each). Per core:
  - phi computed with batch rows on partitions (ACT exp, DVE elementwise).
  - phi transposed to [l, b] layout via PE transposes.
  - The big einsum runs on the PE: per batch row, one_hot chunk [128l, 80v]
    is the stationary operand, the phi column [128l, 1] is the moving
    operand; out [80, 1] accumulates into psum[0:80, b].  The one_hot
    tensor (335 MB total) is streamed once from HBM -> memory bound.
"""

import sys

sys.path.insert(0, "/opt/trn_rl_repo")

import numpy as np

import concourse.bass as bass
import concourse.bacc as bacc
import concourse.tile as tile
from concourse import mybir
from concourse.bass_utils import run_bass_kernel_spmd
from concourse.masks import make_identity

F32 = mybir.dt.float32
I32 = mybir.dt.int32
AF = mybir.ActivationFunctionType
OP = mybir.AluOpType

B, D, K, L, V = 4096, 512, 10, 256, 80
K3 = 3 * K
NCORES = 8
BC = B // NCORES          # 512 rows per core
NT = BC // 128            # 4 batch tiles per core

_CACHE = {}


def build_nc(mode="full", repeat=1):
    do_phase_a = mode in ("full", "phaseA")
    do_phase_b_mm = mode in ("full", "phaseB")
    do_phase_b_dma = mode in ("full", "phaseB", "dmaonly")
    nc = bacc.Bacc(
        "TRN2",
        target_bir_lowering=False,
        debug=False,
        num_devices=NCORES,
    )

    x = nc.dram_tensor("x", [BC, D], F32, kind="ExternalInput").ap()
    pk = nc.dram_tensor("pk", [BC, K], F32, kind="ExternalInput").ap()
    oh = nc.dram_tensor("oh", [BC, L, V], F32, kind="ExternalInput").ap()
    slen = nc.dram_tensor("slen", [BC, 1], I32, kind="ExternalInput").ap()
    Wd = nc.dram_tensor("W", [D, K3], F32, kind="ExternalInput").ap()
    bv = nc.dram_tensor("b", [1, K3], F32, kind="ExternalInput").ap()
    ks = nc.dram_tensor("ks", [1, 1], F32, kind="ExternalInput").ap()
    wT = nc.dram_tensor("wT", [V, BC], F32, kind="ExternalOutput").ap()
    kap = nc.dram_tensor("kap", [BC, K], F32, kind="ExternalOutput").ap()

    with tile.TileContext(nc) as tc:
        with (
            tc.tile_pool(name="singles", bufs=1) as singles,
            tc.tile_pool(name="ptile", bufs=2) as ptile,
            tc.tile_pool(name="small", bufs=2) as small,
            tc.tile_pool(name="kwork", bufs=3) as kwork,
            tc.tile_pool(name="ohp", bufs=12) as ohp,
            tc.tile_pool(name="pst", bufs=2, space="PSUM") as pst,
            tc.tile_pool(name="psr", bufs=2, space="PSUM") as psr,
            tc.tile_pool(name="psw", bufs=1, space="PSUM") as psw,
        ):
            # ---- constants ----
            ident = singles.tile([128, 128], F32)
            make_identity(nc, ident)

            u0i = singles.tile([128, L], I32)
            nc.gpsimd.iota(u0i, pattern=[[1, L]], base=0, channel_multiplier=0)
            u0 = singles.tile([128, L], F32)
            nc.vector.tensor_copy(u0, u0i)  # 0..255 on every partition

            W_sb = singles.tile([128, D // 128, K3], F32)
            nc.sync.dma_start(out=W_sb, in_=Wd.rearrange("(c p) k -> p c k", p=128))

            b_bc = singles.tile([128, K3], F32)
            nc.gpsimd.dma_start(
                out=b_bc,
                in_=bass.AP(tensor=bv.tensor, offset=0, ap=[[0, 128], [1, K3]]),
            )
            ks_col = singles.tile([128, 1], F32)
            nc.gpsimd.dma_start(
                out=ks_col,
                in_=bass.AP(tensor=ks.tensor, offset=0, ap=[[0, 128], [1, 1]]),
            )
            # fold kappa_scale into the kappa_hat slice of the bias
            nc.vector.tensor_scalar(
                out=b_bc[:, 2 * K : 3 * K],
                in0=b_bc[:, 2 * K : 3 * K],
                scalar1=ks_col[:],
                scalar2=None,
                op0=OP.add,
            )

            phiTe = singles.tile([128, BC], F32)  # phi[b, 2p]   (even l)
            phiTo = singles.tile([128, BC], F32)  # phi[b, 2p+1] (odd l)

            def phase_a(t):
                row = slice(t * 128, (t + 1) * 128)

                x_t = ptile.tile([128, D], F32, tag="x_t")
                nc.sync.dma_start(out=x_t, in_=x[row, :])
                pk_t = small.tile([128, K], F32, tag="pk_t")
                nc.sync.dma_start(out=pk_t, in_=pk[row, :])
                len_i = small.tile([128, 1], I32, tag="len_i")
                nc.sync.dma_start(out=len_i, in_=slen[row, :])
                len_f = small.tile([128, 1], F32, tag="len_f")
                nc.vector.tensor_copy(len_f, len_i)

                # x tile transposed: xT[d, b] in 4 chunks of 128
                xT = ptile.tile([128, D // 128, 128], F32, tag="xT")
                for dc in range(D // 128):
                    ps = pst.tile([128, 128], F32, tag="ps_tr")
                    nc.tensor.transpose(ps, x_t[:, dc * 128 : (dc + 1) * 128], ident)
                    nc.vector.tensor_copy(xT[:, dc, :], ps)

                # raw = x @ W  (accumulate over 4 d-chunks)
                ps_raw = psr.tile([128, K3], F32, tag="ps_raw")
                for dc in range(D // 128):
                    nc.tensor.matmul(
                        ps_raw,
                        xT[:, dc, :],
                        W_sb[:, dc, :],
                        start=(dc == 0),
                        stop=(dc == D // 128 - 1),
                    )
                hats = small.tile([128, K3], F32, tag="hats")
                nc.vector.scalar_tensor_tensor(
                    out=hats, in0=ps_raw, scalar=1.0, in1=b_bc,
                    op0=OP.mult, op1=OP.add,
                )

                lnal = small.tile([128, K], F32, tag="lnal")
                nc.vector.tensor_scalar(
                    out=lnal, in0=hats[:, 0:K],
                    scalar1=-8.0, scalar2=8.0, op0=OP.max, op1=OP.min,
                )
                bcl = small.tile([128, K], F32, tag="bcl")
                nc.vector.tensor_scalar(
                    out=bcl, in0=hats[:, K : 2 * K],
                    scalar1=-8.0, scalar2=8.0, op0=OP.max, op1=OP.min,
                )
                beta = small.tile([128, K], F32, tag="beta")
                nc.scalar.activation(out=beta, in_=bcl, func=AF.Exp)
                dcl = small.tile([128, K], F32, tag="dcl")
                nc.vector.tensor_scalar(
                    out=dcl, in0=hats[:, 2 * K : 3 * K],
                    scalar1=-8.0, scalar2=5.0, op0=OP.max, op1=OP.min,
                )
                dk = small.tile([128, K], F32, tag="dk")
                nc.scalar.activation(out=dk, in_=dcl, func=AF.Exp)
                kap_t = small.tile([128, K], F32, tag="kap_t")
                nc.vector.tensor_add(kap_t, pk_t, dk)
                nc.sync.dma_start(out=kap[row, :], in_=kap_t)
                # kap1 = 1 - kappa; then u0 + kap1 = (l+1) - kappa
                kap1 = small.tile([128, K], F32, tag="kap1")
                nc.vector.tensor_scalar(
                    out=kap1, in0=kap_t,
                    scalar1=-1.0, scalar2=1.0, op0=OP.mult, op1=OP.add,
                )

                # e_all[p, l, k] = exp(lnal_k - min(beta_k*(u_l-kappa_k)^2, 50))
                e_all = ptile.tile([128, L, K], F32, tag="e_all")
                for k in range(K):
                    diff = kwork.tile([128, L], F32, tag="diff")
                    nc.vector.tensor_scalar(
                        out=diff, in0=u0, scalar1=kap1[:, k : k + 1],
                        scalar2=None, op0=OP.add,
                    )
                    sq = kwork.tile([128, L], F32, tag="sq")
                    nc.vector.scalar_tensor_tensor(
                        out=sq, in0=diff, scalar=beta[:, k : k + 1], in1=diff,
                        op0=OP.mult, op1=OP.mult,
                    )
                    sc = kwork.tile([128, L], F32, tag="sc")
                    nc.vector.tensor_scalar(
                        out=sc, in0=sq, scalar1=50.0, scalar2=None, op0=OP.min,
                    )
                    nc.scalar.activation(
                        out=e_all[:, :, k], in_=sc, func=AF.Exp,
                        bias=lnal[:, k : k + 1], scale=-1.0,
                    )

                phi_r = kwork.tile([128, L], F32, tag="phi_r")
                nc.vector.tensor_reduce(
                    out=phi_r, in_=e_all, axis=mybir.AxisListType.X, op=OP.add,
                )
                msk = kwork.tile([128, L], F32, tag="msk")
                nc.vector.tensor_scalar(
                    out=msk, in0=u0, scalar1=len_f[:], scalar2=None, op0=OP.is_lt,
                )
                phi_m = kwork.tile([128, L], F32, tag="phi_m")
                nc.vector.tensor_mul(phi_m, phi_r, msk)

                # transpose phi to [l, b]; even/odd split matches the
                # 2-l-per-partition packing of the one_hot stream
                phi2 = phi_m.rearrange("p (l two) -> p two l", two=2)
                pse = pst.tile([128, 128], F32, tag="ps_tr")
                nc.tensor.transpose(pse, phi2[:, 0, :], ident)
                nc.vector.tensor_copy(phiTe[:, row], pse)
                pso = pst.tile([128, 128], F32, tag="ps_tr")
                nc.tensor.transpose(pso, phi2[:, 1, :], ident)
                nc.vector.tensor_copy(phiTo[:, row], pso)

            oh_r = oh.rearrange("b (p two) v -> b p two v", two=2)

            def phase_b(ps_w):
                # w^T[v, b] = sum_l phi[b, l] * oh[b, l, v] on the PE
                for b in range(BC if do_phase_b_dma else 0):
                    oht = ohp.tile([128, 2, V], F32, tag="oht")
                    nc.sync.dma_start(out=oht, in_=oh_r[b])
                    if do_phase_b_mm:
                        nc.tensor.matmul(
                            ps_w[0:V, b : b + 1], oht[:, 0, :],
                            phiTe[:, b : b + 1], start=True, stop=False,
                        )
                        nc.tensor.matmul(
                            ps_w[0:V, b : b + 1], oht[:, 1, :],
                            phiTo[:, b : b + 1], start=False, stop=True,
                        )

            # repeat>1 builds R copies of the compute into one NEFF so the
            # per-iteration HW time can be measured as a wall-clock slope.
            for _rep in range(repeat):
                for t in range(NT if do_phase_a else 0):
                    phase_a(t)
                ps_w = psw.tile([128, BC], F32, tag="ps_w")
                phase_b(ps_w)
                w_sb = singles.tile([V, BC], F32, tag=f"w_sb{_rep}")
                if do_phase_b_mm:
                    nc.vector.tensor_copy(w_sb, ps_w[0:V, :])
                else:
                    nc.vector.memset(w_sb, 0.0)
                nc.sync.dma_start(out=wT, in_=w_sb)

    nc.compile()
    return nc


def _get_nc():
    if "nc" not in _CACHE:
        _CACHE["nc"] = build_nc()
    return _CACHE["nc"]


def kernel(inputs, prev_kappa, char_seq_one_hot, sequence_lengths, W, b,
           kappa_scale):
    inputs = np.asarray(inputs, dtype=np.float32)
    prev_kappa = np.asarray(prev_kappa, dtype=np.float32)
    char_seq_one_hot = np.asarray(char_seq_one_hot, dtype=np.float32)
    sequence_lengths = np.asarray(sequence_lengths, dtype=np.int32)
    W = np.ascontiguousarray(np.asarray(W, dtype=np.float32))
    b_arr = np.ascontiguousarray(np.asarray(b, dtype=np.float32).reshape(1, K3))
    ks_arr = np.ascontiguousarray(
        np.asarray(kappa_scale, dtype=np.float32).reshape(1, 1)
    )

    nc = _get_nc()
    in_maps = []
    for c in range(NCORES):
        sl = slice(c * BC, (c + 1) * BC)
        in_maps.append({
            "x": np.ascontiguousarray(inputs[sl]),
            "pk": np.ascontiguousarray(prev_kappa[sl]),
            "oh": np.ascontiguousarray(char_seq_one_hot[sl]),
            "slen": np.ascontiguousarray(sequence_lengths[sl].reshape(BC, 1)),
            "W": W,
            "b": b_arr,
            "ks": ks_arr,
        })

    res = run_bass_kernel_spmd(nc, in_maps, core_ids=list(range(NCORES)))
    w = np.concatenate([r["wT"].T for r in res.results], axis=0)
    kappa = np.concatenate([r["kap"] for r in res.results], axis=0)
    return (w, kappa)
